# revision 8
# baseline (speedup 1.0000x reference)
"""MoE ExpertsLayer kernel for 8 TRN2 NeuronCores (Bass/Tile, SPMD).

Strategy (expert-parallel, per the sharding hint):
  - Host computes the router (tiny: [8192,1024]@[1024,8]) in fp64 to build
    the top-2 dispatch: per-expert token index lists + combine weights.
  - Core e receives the tokens routed to expert e (gathered, transposed,
    zero-padded to capacity C), that expert's SwiGLU weights, plus a
    1/8 data-parallel slice of all tokens for the shared expert + router
    gate_vals output.
  - On device, everything is fp32-stored / float32r tensor-engine compute
    (full PE speed at free-dim>=256, ~1e-4 relative error):
      expert:  h = silu(x@Wg) * (x@Wu)  (spilled to DRAM),  y = (h@Wd) * cw
      shared:  gate_vals slice, sig = sigmoid(x@seg) (broadcast via DMA),
               hs = silu(x@Sg) * (x@Su),  ys = (hs@Sd) * sig
    All activations live transposed ([feature, token]) so every GEMM maps
    to matmul(psum, lhsT=weight_tile[K,128], rhs=actT[K, tokens]) with
    natural-layout weights and no on-device transposes.
  - Host scatter-adds the two expert contributions per token + shared.

Toolchain constraint: walrus in this image rejects >1 semaphore wait on
most instruction structs, so after Tile scheduling we run a post-pass that
hoists excess waits onto same-engine NoOp carriers.
"""

import os
import sys

for _p in ("/opt/trn_rl_repo", "/root/.axon_site/_ro/trn_rl_repo"):
    if os.path.isdir(_p) and _p not in sys.path:
        sys.path.insert(0, _p)

from contextlib import ExitStack

import numpy as np

import concourse.bass as bass
import concourse.tile as tile
from concourse import mybir

F32 = mybir.dt.float32
F32R = mybir.dt.float32r
AF = mybir.ActivationFunctionType

D = 1024          # hidden dim
I = 2048          # expert intermediate dim
E = 8             # experts == cores
TOK = 8192        # total tokens
TS = TOK // 8     # tokens per core for shared expert / router
C = 2304          # per-expert token capacity (max real count is 2182)
EBLOCKS = [512, 512, 512, 512, 256]   # token blocks for the expert pass
SBLOCKS = [512, 512]                  # token blocks for the shared pass
KD = D // 128     # k-chunks when contracting over D
KI = I // 128     # k-chunks when contracting over I
NI = I // 128     # i-tiles (output rows of gate/up)
ND = D // 128     # d-tiles (output rows of down)


def _split_excess_waits(nc, cap=1):
    """walrus (this image) rejects >cap sync waits on most instruction
    structs; hoist extras onto same-engine NoOps placed just before."""
    n_split = 0
    for f in nc.m.functions:
        for blk in f.blocks:
            insts = list(blk.instructions)
            out = []
            changed = False
            for inst in insts:
                si = inst.sync_info
                waits = list(si.on_wait) if si is not None else []
                if len(waits) > cap:
                    keep = waits[-cap:]
                    for j, w in enumerate(waits[:-cap]):
                        nop = mybir.InstNoOp(
                            name=f"{inst.name}-ws{j}", ins=[], outs=[]
                        )
                        nop.engine = inst.engine
                        nop.sync_info = mybir.SyncInfo(on_wait=[w], on_update=[])
                        out.append(nop)
                        n_split += 1
                    inst.sync_info = mybir.SyncInfo(
                        on_wait=keep, on_update=list(si.on_update)
                    )
                    changed = True
                out.append(inst)
            if changed:
                blk.instructions = out
    return n_split


def _bcast_ap(src_row_ap, parts=128):
    """AP that reads a [1, N] region replicated across `parts` partitions."""
    return bass.AP(
        tensor=src_row_ap.tensor,
        offset=src_row_ap.offset,
        ap=[[0, parts]] + list(src_row_ap.ap[1:]),
    )


def _emit_gate_up(nc, tc, ctx, wg_src, wu_src, get_x, h_dst, blocks, name):
    """h_dst[i, t] = silu(x@Wg)^T * (x@Wu)^T, streamed per token block."""
    wp = ctx.enter_context(tc.tile_pool(name=f"{name}_w", bufs=1))
    sp = ctx.enter_context(tc.tile_pool(name=f"{name}_s", bufs=1))
    ps = tc._moe_psum_pool
    wg_t = []
    wu_t = []
    for k in range(KD):
        t = wp.tile([128, I], F32R, tag=f"wg{k}")
        nc.sync.dma_start(out=t, in_=wg_src[k * 128:(k + 1) * 128, :])
        wg_t.append(t)
        t = wp.tile([128, I], F32R, tag=f"wu{k}")
        nc.sync.dma_start(out=t, in_=wu_src[k * 128:(k + 1) * 128, :])
        wu_t.append(t)
    off = 0
    for tb in blocks:
        xb = get_x(off, tb)
        for i in range(NI):
            gp = ps.tile([128, tb], F32, tag="mm", bufs=4)
            up = ps.tile([128, tb], F32, tag="mm", bufs=4)
            isl = bass.ts(i, 128)
            for k in range(KD):
                nc.tensor.matmul(gp, wg_t[k][:, isl], xb[k],
                                 start=(k == 0), stop=(k == KD - 1))
            for k in range(KD):
                nc.tensor.matmul(up, wu_t[k][:, isl], xb[k],
                                 start=(k == 0), stop=(k == KD - 1))
            sg = sp.tile([128, tb], F32R, tag="sg", bufs=3)
            nc.scalar.activation(out=sg, in_=gp, func=ACT_SILU)
            us = sp.tile([128, tb], F32R, tag="us", bufs=3)
            nc.scalar.copy(out=us, in_=up)
            ht = sp.tile([128, tb], F32R, tag="h", bufs=3)
            nc.vector.tensor_mul(ht, sg, us)
            nc.sync.dma_start(out=h_dst[isl, off:off + tb], in_=ht)
        off += tb


def _emit_down(nc, tc, ctx, wd_src, h_src, scale_sb, out_dst, blocks, name):
    """out_dst[d, t] = (h^T @ Wd)^T[d, t] * scale_sb[*, t]."""
    wp = ctx.enter_context(tc.tile_pool(name=f"{name}_w", bufs=1))
    sp = ctx.enter_context(tc.tile_pool(name=f"{name}_s", bufs=1))
    ps = tc._moe_psum_pool
    wd_t = []
    for k in range(KI):
        t = wp.tile([128, D], F32R, tag=f"wd{k}")
        nc.sync.dma_start(out=t, in_=wd_src[k * 128:(k + 1) * 128, :])
        wd_t.append(t)
    off = 0
    for tb in blocks:
        hb = []
        for k in range(KI):
            t = sp.tile([128, tb], F32R, tag=f"hb{k}", bufs=2)
            nc.sync.dma_start(out=t, in_=h_src[k * 128:(k + 1) * 128,
                                              off:off + tb])
            hb.append(t)
        for d in range(ND):
            zp = ps.tile([128, tb], F32, tag="mm", bufs=4)
            dsl = bass.ts(d, 128)
            for k in range(KI):
                nc.tensor.matmul(zp, wd_t[k][:, dsl], hb[k],
                                 start=(k == 0), stop=(k == KI - 1))
            zt = sp.tile([128, tb], F32, tag="z", bufs=3)
            nc.vector.tensor_mul(zt, zp, scale_sb[:, off:off + tb])
            nc.sync.dma_start(out=out_dst[dsl, off:off + tb], in_=zt)
        off += tb


ACT_SILU = AF.Silu  # swapped to Tanh for CoreSim validation (no Silu in sim)
PHASES = {"eg", "ed", "sv", "sg", "sd"}  # build subsets for HW bisection


def build_nc(split_waits=True):
    nc = bass.Bass()
    xeT = nc.declare_dram_parameter("xeT", [D, C], F32R, isOutput=False)
    cw = nc.declare_dram_parameter("cw", [1, C], F32, isOutput=False)
    xsT = nc.declare_dram_parameter("xsT", [D, TS], F32R, isOutput=False)
    wg = nc.declare_dram_parameter("wg", [D, I], F32R, isOutput=False)
    wu = nc.declare_dram_parameter("wu", [D, I], F32R, isOutput=False)
    wd = nc.declare_dram_parameter("wd", [I, D], F32R, isOutput=False)
    sgw = nc.declare_dram_parameter("sgw", [D, I], F32R, isOutput=False)
    suw = nc.declare_dram_parameter("suw", [D, I], F32R, isOutput=False)
    sdw = nc.declare_dram_parameter("sdw", [I, D], F32R, isOutput=False)
    gww = nc.declare_dram_parameter("gww", [D, E], F32R, isOutput=False)
    segw = nc.declare_dram_parameter("segw", [D, 1], F32R, isOutput=False)
    yeT = nc.declare_dram_parameter("yeT", [D, C], F32, isOutput=True)
    ysT = nc.declare_dram_parameter("ysT", [D, TS], F32, isOutput=True)
    gvT = nc.declare_dram_parameter("gvT", [E, TS], F32, isOutput=True)

    with tile.TileContext(nc) as tc, ExitStack() as ctx:
        ps = ctx.enter_context(tc.tile_pool(name="ps", bufs=1, space="PSUM"))
        tc._moe_psum_pool = ps
        dr = ctx.enter_context(tc.tile_pool(name="dram", bufs=1, space="DRAM"))
        hT = dr.tile([I, C], F32R)
        hsT = dr.tile([I, TS], F32R)
        sigd = dr.tile([1, TS], F32)

        # ---- expert pass ----
        if "eg" in PHASES:
          with ExitStack() as s1:
            xp = s1.enter_context(tc.tile_pool(name="xe", bufs=1))

            def get_xe(off, tb):
                out = []
                for k in range(KD):
                    t = xp.tile([128, tb], F32R, tag=f"x{k}", bufs=2)
                    nc.sync.dma_start(
                        out=t, in_=xeT[k * 128:(k + 1) * 128, off:off + tb])
                    out.append(t)
                return out

            _emit_gate_up(nc, tc, s1, wg, wu, get_xe, hT, EBLOCKS, "eg")
        if "ed" in PHASES:
          with ExitStack() as s2:
            cp = s2.enter_context(tc.tile_pool(name="cw", bufs=1))
            cwb = cp.tile([128, C], F32)
            nc.sync.dma_start(out=cwb, in_=_bcast_ap(cw[0:1, :]))
            _emit_down(nc, tc, s2, wd, hT, cwb, yeT, EBLOCKS, "ed")

        # ---- shared pass: router gate_vals + sigmoid gate first ----
        if "sv" in PHASES or "sg" in PHASES:
          with ExitStack() as s3:
            xsp = s3.enter_context(tc.tile_pool(name="xs", bufs=1))
            xst = []
            for k in range(KD):
                t = xsp.tile([128, TS], F32R, tag=f"xs{k}")
                nc.sync.dma_start(out=t, in_=xsT[k * 128:(k + 1) * 128, :])
                xst.append(t)
            smp = s3.enter_context(tc.tile_pool(name="sm", bufs=1))
            if "sv" not in PHASES:
                pass
            gwt = smp.tile([128, KD, E], F32R)
            nc.sync.dma_start(out=gwt,
                              in_=gww.rearrange("(k p) e -> p k e", p=128))
            segt = smp.tile([128, KD, 1], F32R)
            nc.sync.dma_start(out=segt,
                              in_=segw.rearrange("(k p) e -> p k e", p=128))
            for n in range(TS // 512 if "sv" in PHASES else 0):
                nsl = bass.ts(n, 512)
                gvp = ps.tile([E, 512], F32, tag="gv", bufs=1)
                for k in range(KD):
                    nc.tensor.matmul(gvp, gwt[:, k, :], xst[k][:, nsl],
                                     start=(k == 0), stop=(k == KD - 1))
                gvs = smp.tile([E, 512], F32, tag="gvs", bufs=2)
                nc.scalar.copy(out=gvs, in_=gvp)
                nc.sync.dma_start(out=gvT[:, nsl], in_=gvs)
                sip = ps.tile([1, 512], F32, tag="sig", bufs=1)
                for k in range(KD):
                    nc.tensor.matmul(sip, segt[:, k, :], xst[k][:, nsl],
                                     start=(k == 0), stop=(k == KD - 1))
                sis = smp.tile([1, 512], F32, tag="sis", bufs=2)
                nc.scalar.activation(out=sis, in_=sip, func=AF.Sigmoid)
                nc.sync.dma_start(out=sigd[0:1, nsl], in_=sis)

            def get_xs(off, tb):
                return [t[:, off:off + tb] for t in xst]

            if "sg" in PHASES:
                _emit_gate_up(nc, tc, s3, sgw, suw, get_xs, hsT, SBLOCKS, "sg")
        if "sd" in PHASES:
          with ExitStack() as s4:
            sbp = s4.enter_context(tc.tile_pool(name="sb4", bufs=1))
            sigb = sbp.tile([128, TS], F32)
            nc.sync.dma_start(out=sigb, in_=_bcast_ap(sigd[0:1, :]))
            _emit_down(nc, tc, s4, sdw, hsT, sigb, ysT, SBLOCKS, "sd")

    if split_waits:
        _split_excess_waits(nc)
    return nc


# ---------------------------------------------------------------------------
# host side
# ---------------------------------------------------------------------------

_STATE = {}


def _get_runner():
    """Compile once; return a callable(in_maps) -> list[dict] (per core)."""
    if "run" in _STATE:
        return _STATE["run"]

    import jax
    from jax.experimental.shard_map import shard_map
    from jax.sharding import Mesh, PartitionSpec

    from concourse import bass2jax
    from concourse import mybir as _mybir

    nc = build_nc()
    bass2jax.install_neuronx_cc_hook()

    n_cores = 8
    partition_name = (nc.partition_id_tensor.name
                      if nc.partition_id_tensor else None)
    in_names, out_names, out_avals, zero_shapes = [], [], [], []
    for alloc in nc.m.functions[0].allocations:
        if not isinstance(alloc, _mybir.MemoryLocationSet):
            continue
        name = alloc.memorylocations[0].name
        if alloc.kind == "ExternalInput":
            if name != partition_name:
                in_names.append(name)
        elif alloc.kind == "ExternalOutput":
            shape = tuple(alloc.tensor_shape)
            dtype = _mybir.dt.np(alloc.dtype)
            out_names.append(name)
            out_avals.append(jax.core.ShapedArray(shape, dtype))
            zero_shapes.append((shape, dtype))
    n_params = len(in_names)
    n_outs = len(out_avals)
    all_in_names = list(in_names) + list(out_names)
    if partition_name is not None:
        all_in_names.append(partition_name)

    def _body(*args):
        operands = list(args)
        if partition_name is not None:
            operands.append(bass2jax.partition_id_tensor())
        outs = bass2jax._bass_exec_p.bind(
            *operands,
            out_avals=tuple(out_avals),
            in_names=tuple(all_in_names),
            out_names=tuple(out_names),
            lowering_input_output_aliases=(),
            sim_require_finite=True,
            sim_require_nnan=True,
            nc=nc,
        )
        return tuple(outs)

    devices = jax.devices()[:n_cores]
    mesh = Mesh(np.asarray(devices), ("core",))
    in_specs = (PartitionSpec("core"),) * (n_params + n_outs)
    out_specs = (PartitionSpec("core"),) * n_outs
    donate = tuple(range(n_params, n_params + n_outs))
    sharded = jax.jit(
        shard_map(_body, mesh=mesh, in_specs=in_specs, out_specs=out_specs,
                  check_rep=False),
        donate_argnums=donate, keep_unused=True,
    )

    def run(in_maps):
        concat_in = [
            np.concatenate([np.asarray(in_maps[c][nm]) for c in range(n_cores)],
                           axis=0)
            for nm in in_names
        ]
        concat_zeros = [
            np.zeros((n_cores * s[0], *s[1:]), dt) for (s, dt) in zero_shapes
        ]
        out_arrs = sharded(*concat_in, *concat_zeros)
        return [
            {nm: np.asarray(out_arrs[i]).reshape(n_cores, *zero_shapes[i][0])[c]
             for i, nm in enumerate(out_names)}
            for c in range(n_cores)
        ]

    _STATE["run"] = run
    return run


def _route(x, gate_w):
    """Top-2 routing in fp64 (min top2/top3 prob gap is ~1e-5, far above
    fp32-vs-fp64 rounding noise, so this matches the fp32 reference)."""
    gv = x.astype(np.float64) @ gate_w.astype(np.float64)
    gv -= gv.max(axis=-1, keepdims=True)
    p = np.exp(gv)
    p /= p.sum(axis=-1, keepdims=True)
    top2 = np.argsort(-p, axis=-1)[:, :2]
    return p, top2


def _numpy_fallback(x, gate_w, egw, euw, edw, sgp, suw, sdw, seg):
    """Dense fp32 fallback (only if a capacity overflow ever happened)."""
    gv = (x @ gate_w).astype(np.float32)
    p, top2 = _route(x, gate_w)
    T = x.shape[0]
    dense_w = np.zeros((T, E), np.float32)
    dense_w[np.arange(T)[:, None], top2] = p[np.arange(T)[:, None], top2]
    out = np.zeros((T, D), np.float32)
    for e in range(E):
        g = x @ egw[e]
        u = x @ euw[e]
        h = (g / (1 + np.exp(-g))) * u
        out += dense_w[:, e:e + 1] * (h @ edw[e])
    g = x @ sgp
    u = x @ suw
    sh = ((g / (1 + np.exp(-g))) * u) @ sdw
    out += (1 / (1 + np.exp(-(x @ seg)))) * sh
    return out, gv


def _prepare(hidden_states, gate_w, expert_gate_w, expert_up_w, expert_down_w,
             shared_gate_proj_w, shared_up_w, shared_down_w,
             shared_expert_gate_w, top_k):
    assert int(top_k) == 2
    x = np.ascontiguousarray(np.asarray(hidden_states, np.float32)
                             .reshape(TOK, D))
    gate_w = np.asarray(gate_w, np.float32)
    egw = np.asarray(expert_gate_w, np.float32)
    euw = np.asarray(expert_up_w, np.float32)
    edw = np.asarray(expert_down_w, np.float32)
    sgp = np.asarray(shared_gate_proj_w, np.float32)
    suw = np.asarray(shared_up_w, np.float32)
    sdw = np.asarray(shared_down_w, np.float32)
    seg = np.asarray(shared_expert_gate_w, np.float32)

    p, top2 = _route(x, gate_w)
    sels, cws = [], []
    for e in range(E):
        sel = np.flatnonzero((top2 == e).any(axis=-1))
        if len(sel) > C:
            return None  # capacity overflow -> numpy fallback
        sels.append(sel)
        cws.append(p[sel, e].astype(np.float32))

    xT = np.ascontiguousarray(x.T)
    in_maps = []
    for e in range(E):
        sel = sels[e]
        xeT = np.zeros((D, C), np.float32)
        xeT[:, :len(sel)] = xT[:, sel]
        cw = np.zeros((1, C), np.float32)
        cw[0, :len(sel)] = cws[e]
        in_maps.append({
            "xeT": xeT,
            "cw": cw,
            "xsT": np.ascontiguousarray(xT[:, e * TS:(e + 1) * TS]),
            "wg": np.ascontiguousarray(egw[e]),
            "wu": np.ascontiguousarray(euw[e]),
            "wd": np.ascontiguousarray(edw[e]),
            "sgw": sgp, "suw": suw, "sdw": sdw,
            "gww": gate_w,
            "segw": seg,
        })
    return {"in_maps": in_maps, "sels": sels}


def _combine(results, sels):
    final = np.empty((TOK, D), np.float32)
    for c in range(E):
        final[c * TS:(c + 1) * TS] = results[c]["ysT"].T
    for e in range(E):
        final[sels[e]] += results[e]["yeT"].T[:len(sels[e])]
    gate_vals = np.concatenate([results[c]["gvT"].T for c in range(E)], axis=0)
    return final, gate_vals


def kernel(hidden_states, gate_w, expert_gate_w, expert_up_w, expert_down_w,
           shared_gate_proj_w, shared_up_w, shared_down_w,
           shared_expert_gate_w, top_k):
    B, S, _ = hidden_states.shape
    prep = _prepare(hidden_states, gate_w, expert_gate_w, expert_up_w,
                    expert_down_w, shared_gate_proj_w, shared_up_w,
                    shared_down_w, shared_expert_gate_w, top_k)
    if prep is None:
        x = np.asarray(hidden_states, np.float32).reshape(TOK, D)
        out, gv = _numpy_fallback(
            x, np.asarray(gate_w, np.float32),
            np.asarray(expert_gate_w, np.float32),
            np.asarray(expert_up_w, np.float32),
            np.asarray(expert_down_w, np.float32),
            np.asarray(shared_gate_proj_w, np.float32),
            np.asarray(shared_up_w, np.float32),
            np.asarray(shared_down_w, np.float32),
            np.asarray(shared_expert_gate_w, np.float32))
        return out.reshape(B, S, D), gv

    run = _get_runner()
    results = run(prep["in_maps"])
    final, gate_vals = _combine(results, prep["sels"])
    return final.reshape(B, S, D), gate_vals
